# revision 40
# baseline (speedup 1.0000x reference)
"""Distributed causal attention w/ RoPE for TRN2 (8 NeuronCores).

Sharding: tensor-parallel over heads (2 heads/core) for QKV+attention,
sequence-parallel for the output projection via AllToAll (each core
projects a disjoint 256-token slice per batch with the full w_proj),
so no cross-core reduction is needed.

Per core:
  - QKV projection in transposed layout (features on partitions) for its
    2 heads; RoPE via pair-swapped strided-DMA copies + DVE mul/add.
  - V transposed to [tok, d] via PE transposes (both heads per 128-tok
    block in one transpose), augmented with a ones column so the AV
    matmul (M=65) also produces the softmax denominators.
  - Scores computed transposed [tk, tq] with the two heads' K=64 matmuls
    row-packed (base partitions 0/64) so they run concurrently; exp on
    the Scalar engine over paired [128,1024] PSUM tiles; causal diagonal
    handled by column-restricted matmuls + a triangular mask multiply.
  - Normalization: reciprocal of the denominator row, broadcast across
    64 partitions with a K=1 matmul pair (col-packed), then one
    scalar_tensor_tensor per head.
  - Per-batch AllToAll ships y head-shards to token-slice owners; each
    core projects its 2x256 tokens with the full (replicated) w_proj.
Host side: input layout prep (transposes/permutes) and concatenation of
disjoint per-core token slices of the final output.
"""

import numpy as np

import concourse.bass as bass
import concourse.bacc as bacc
import concourse.mybir as mybir
from concourse import tile
from concourse.bass_utils import run_bass_kernel_spmd

B, T, C, H, D = 2, 2048, 1024, 16, 64
NCORE = 8
HPC = H // NCORE          # heads per core = 2
TCH = 512                 # query group width
NTC = T // TCH            # 4
NBLK = T // 128           # 16 tk blocks
TOK = T // NCORE          # 256 tokens per core per batch (proj slice)
ROPE_BASE = 10000.0
F32 = mybir.dt.float32
F16 = mybir.dt.float16


def _rope_tables():
    # row p of a q/k tile holds head_local = p // 64, d = p % 64
    d = np.arange(D)
    j = d // 2
    theta = ROPE_BASE ** (-(2.0 * j) / D)          # per-row theta
    t = np.arange(T, dtype=np.float64)
    ang = t[None, :] * theta[:, None]              # [64, T]
    cos = np.cos(ang)
    sin = np.sin(ang)
    sgn = np.where(d % 2 == 0, -1.0, 1.0)[:, None]
    c1 = np.concatenate([cos, cos], axis=0)        # [128, T]
    s1 = np.concatenate([sgn * sin, sgn * sin], axis=0)
    scale = 1.0 / np.sqrt(D)
    return (
        (c1 * scale).astype(np.float16),
        (s1 * scale).astype(np.float16),
        c1.astype(np.float16),
        s1.astype(np.float16),
    )


def build(debug=False):
    nc = bacc.Bacc(num_devices=NCORE)
    # x2[b*4+g, p, c*512+t] = x[b, g*512+t, c*128+p]
    x2 = nc.declare_dram_parameter("x2", [B * NTC, 128, 4096], F16,
                                   isOutput=False)
    # w2[p, c*384+m] = w_qkv_perm[m, c*128+p]  (m: q0..q127,k0..,v0..)
    w2 = nc.declare_dram_parameter("w2", [128, 8 * 384], F16, isOutput=False)
    # wp2[p, c*1024+m] = w_proj[m, c*128+p]
    wp2 = nc.declare_dram_parameter("wp2", [128, 8 * 1024], F16,
                                    isOutput=False)
    # out[b*8+o, p, t] = out_feature(o*128+p) of token (my_slice, t)
    out_ext = nc.declare_dram_parameter("out", [B * 8, 128, TOK], F16,
                                        isOutput=True)

    cq_np, sq_np, ck_np, sk_np = _rope_tables()
    tri_np = (np.arange(128)[None, :] >= np.arange(128)[:, None])
    cq_c = nc.inline_tensor(cq_np, name="cq")
    sq_c = nc.inline_tensor(sq_np, name="sq")
    ck_c = nc.inline_tensor(ck_np, name="ck")
    sk_c = nc.inline_tensor(sk_np, name="sk")
    tri_c = nc.inline_tensor(tri_np.astype(np.float16), name="tri")
    ident_c = nc.inline_tensor(np.eye(128, dtype=np.float16), name="ident")
    ones2_np = np.zeros((2, 128), np.float16)
    ones2_np[0, 0:64] = 1.0
    ones2_np[1, 64:128] = 1.0
    ones2_c = nc.inline_tensor(ones2_np, name="ones2")

    cc_in = [nc.dram_tensor(f"cc_in{b}", [C, TOK], F16) for b in range(B)]
    cc_out = [nc.dram_tensor(f"cc_out{b}", [C, TOK], F16) for b in range(B)]
    groups = [list(range(NCORE))]
    dbg = {}
    if debug:
        dbg["rope_q"] = nc.declare_dram_parameter("dbg_rope_q", [128, T], F16,
                                                  isOutput=True)
        dbg["rope_k"] = nc.declare_dram_parameter("dbg_rope_k", [128, T], F16,
                                                  isOutput=True)
        dbg["vaug"] = nc.declare_dram_parameter("dbg_vaug", [128, NBLK * 65],
                                                F16, isOutput=True)
        dbg["e0"] = nc.declare_dram_parameter("dbg_e0", [128, 1024], F16,
                                              isOutput=True)
        dbg["y0"] = nc.declare_dram_parameter("dbg_y0", [65, TCH], F32,
                                              isOutput=True)
        dbg["rbc"] = nc.declare_dram_parameter("dbg_rbc", [128, TCH], F32,
                                               isOutput=True)
        dbg["ypair"] = nc.declare_dram_parameter("dbg_ypair", [128, TCH], F16,
                                                 isOutput=True)
        dbg["ya"] = nc.declare_dram_parameter("dbg_ya", [128, 8 * TOK], F16,
                                              isOutput=True)

    with tile.TileContext(nc) as tc:
        with (
            tc.tile_pool(name="const", bufs=1) as cpool,
            tc.tile_pool(name="big", bufs=2) as bpool,
            tc.tile_pool(name="xt", bufs=2) as xpool,
            tc.tile_pool(name="tmp", bufs=2) as tpool,
            tc.tile_pool(name="exp", bufs=4) as epool,
            tc.tile_pool(name="ysmall", bufs=2) as spool,
            tc.tile_pool(name="sc", bufs=2, space="PSUM") as scpool,
            tc.tile_pool(name="ypsum", bufs=2, space="PSUM") as ypool,
            tc.tile_pool(name="mm", bufs=2, space="PSUM") as mmpool,
        ):
            # ---- persistent SBUF loads --------------------------------
            # weights on sync queue (needed first), tables on scalar queue
            w_sb = cpool.tile([128, 8 * 384], F16, tag="w")
            nc.sync.dma_start(out=w_sb[:, :], in_=w2[:, :])
            ident_sb = cpool.tile([128, 128], F16, tag="ident")
            nc.sync.dma_start(out=ident_sb[:, :], in_=ident_c[:, :])
            tri_sb = cpool.tile([128, 128], F16, tag="tri")
            nc.sync.dma_start(out=tri_sb[:, :], in_=tri_c[:, :])
            ones2_sb = cpool.tile([2, 128], F16, tag="ones2")
            nc.sync.dma_start(out=ones2_sb[:, :], in_=ones2_c[:, :])
            cq_sb = cpool.tile([128, T], F16, tag="cq")
            nc.scalar.dma_start(out=cq_sb[:, :], in_=cq_c[:, :])
            sq_sb = cpool.tile([128, T], F16, tag="sq")
            nc.scalar.dma_start(out=sq_sb[:, :], in_=sq_c[:, :])
            ck_sb = cpool.tile([128, T], F16, tag="ck")
            nc.scalar.dma_start(out=ck_sb[:, :], in_=ck_c[:, :])
            sk_sb = cpool.tile([128, T], F16, tag="sk")
            nc.scalar.dma_start(out=sk_sb[:, :], in_=sk_c[:, :])
            wp_sb = cpool.tile([128, 8 * 1024], F16, tag="wp")
            nc.scalar.dma_start(out=wp_sb[:, :], in_=wp2[:, :])
            # prime the ACT exp table load (~2.7us) during the QKV phase
            warm = cpool.tile([1, 8], F32, tag="warm")
            nc.vector.memset(warm[:, :], 0.0)
            nc.scalar.activation(warm[:, :], warm[:, :],
                                 mybir.ActivationFunctionType.Exp)

            # per-b state tiles (bufs=2 so b0/b1 pipeline)
            state = {}

            def qkv_chunk(b, g):
                """QKV projection + RoPE for tokens [g*512,(g+1)*512) of b."""
                rope_q, rope_k, vT = state[b]
                t0 = g * TCH
                xt = xpool.tile([128, 4096], F16, tag="xt")
                nc.sync.dma_start(out=xt[:, :], in_=x2[b * NTC + g, :, :])
                pss = []
                for m in range(3):  # q, k, v
                    p = mmpool.tile([128, TCH], F32, tag="mm")
                    for c in range(8):
                        nc.tensor.matmul(
                            p[:, :],
                            w_sb[:, c * 384 + m * 128:c * 384 + (m + 1) * 128],
                            xt[:, c * TCH:(c + 1) * TCH],
                            start=(c == 0), stop=(c == 7))
                    pss.append(p)
                    if m == 0:
                        q_sb = tpool.tile([128, TCH], F16, tag="qsb")
                        nc.vector.tensor_copy(q_sb[:, :], p[:, :])
                    elif m == 1:
                        k_sb = tpool.tile([128, TCH], F16, tag="ksb")
                        nc.vector.tensor_copy(k_sb[:, :], p[:, :])
                    else:
                        nc.vector.tensor_copy(vT[:, t0:t0 + TCH], p[:, :])
                # pair-swapped copies via strided DMA
                qs_sb = tpool.tile([128, TCH], F16, tag="qssb")
                nc.sync.dma_start(out=qs_sb[0::2, :], in_=q_sb[1::2, :])
                nc.sync.dma_start(out=qs_sb[1::2, :], in_=q_sb[0::2, :])
                ks_sb = tpool.tile([128, TCH], F16, tag="kssb")
                nc.sync.dma_start(out=ks_sb[0::2, :], in_=k_sb[1::2, :])
                nc.sync.dma_start(out=ks_sb[1::2, :], in_=k_sb[0::2, :])
                # rope_q = q*cq + qs*sq ; rope_k = k*ck + ks*sk
                t1 = tpool.tile([128, TCH], F16, tag="t1")
                nc.vector.tensor_mul(t1[:, :], q_sb[:, :],
                                     cq_sb[:, t0:t0 + TCH])
                t2 = tpool.tile([128, TCH], F16, tag="t2")
                nc.vector.tensor_mul(t2[:, :], qs_sb[:, :],
                                     sq_sb[:, t0:t0 + TCH])
                nc.vector.tensor_add(rope_q[:, t0:t0 + TCH], t1[:, :],
                                     t2[:, :])
                t3 = tpool.tile([128, TCH], F16, tag="t1")
                nc.vector.tensor_mul(t3[:, :], k_sb[:, :],
                                     ck_sb[:, t0:t0 + TCH])
                t4 = tpool.tile([128, TCH], F16, tag="t2")
                nc.vector.tensor_mul(t4[:, :], ks_sb[:, :],
                                     sk_sb[:, t0:t0 + TCH])
                nc.vector.tensor_add(rope_k[:, t0:t0 + TCH], t3[:, :],
                                     t4[:, :])

            def transposes(b, g):
                """V -> [tok, d] for the 4 tk-blocks of chunk g (both heads
                in one transpose per block)."""
                _, _, vT = state[b]
                va0, va1 = state[(b, "vaug")]
                for Tt in range(4 * g, 4 * g + 4):
                    tp = mmpool.tile([128, 128], F16, tag="mm")
                    nc.tensor.transpose(
                        tp[:, :], vT[:, Tt * 128:(Tt + 1) * 128],
                        ident_sb[:, :])
                    nc.vector.tensor_copy(va0[:, Tt * 65:Tt * 65 + 64],
                                          tp[:, 0:64])
                    nc.vector.tensor_copy(va1[:, Tt * 65:Tt * 65 + 64],
                                          tp[:, 64:128])

            def emit_sc(b, g, i):
                """Scores (both heads row-packed) + exp (+mask) for tk
                block i of query group g. Returns (c0, e_tile)."""
                rope_q, rope_k, _ = state[b]
                q0 = g * TCH
                diag = (i >= 4 * g)
                c0 = 128 * (i - 4 * g) if diag else 0
                sc = scpool.tile([128, 2, TCH], F32, tag="sc")
                nc.tensor.matmul(
                    sc[:, 0, c0:TCH],
                    rope_k[0:64, i * 128:(i + 1) * 128],
                    rope_q[0:64, q0 + c0:q0 + TCH],
                    start=True, stop=True)
                nc.tensor.matmul(
                    sc[:, 1, c0:TCH],
                    rope_k[64:128, i * 128:(i + 1) * 128],
                    rope_q[64:128, q0 + c0:q0 + TCH],
                    start=True, stop=True)
                e = epool.tile([128, 2, TCH], F16, tag="e")
                # one (possibly strided) exp instruction covers both heads'
                # live column ranges
                nc.scalar.activation(e[:, :, c0:TCH], sc[:, :, c0:TCH],
                                     mybir.ActivationFunctionType.Exp)
                if diag:
                    nc.vector.tensor_mul(e[:, 0, c0:c0 + 128],
                                         e[:, 0, c0:c0 + 128], tri_sb[:, :])
                    nc.vector.tensor_mul(e[:, 1, c0:c0 + 128],
                                         e[:, 1, c0:c0 + 128], tri_sb[:, :])
                if debug and b == 0 and g == 0 and i == 0:
                    nc.sync.dma_start(out=dbg["e0"][:, :],
                                      in_=e[:, :, :])
                return c0, e

            def emit_av(b, g, i, c0, e, y0, y1):
                va0, va1 = state[(b, "vaug")]
                ntk = 4 * g + 4
                first, last = (i == 0), (i == ntk - 1)
                nc.tensor.matmul(
                    y0[:, c0:512], va0[:, i * 65:i * 65 + 65],
                    e[:, 0, c0:TCH],
                    start=first, stop=last, skip_group_check=True)
                nc.tensor.matmul(
                    y1[:, c0:512], va1[:, i * 65:i * 65 + 65],
                    e[:, 1, c0:TCH],
                    start=first, stop=last, skip_group_check=True)

            def finish_group(b, g, fin):
                """Recip of denominators for (b,g); returns finalize closure
                that must be emitted a bit later (after next group's first
                score pairs) to hide the DVE->PE round trip."""
                y0, y1 = fin
                # copy accumulators to SBUF right away: frees the y PSUM
                # slots for the next group, and makes every later consumer
                # (custom-DVE recip included) same-engine ordered on DVE
                ycp0 = spool.tile([65, TCH], F32, tag="ycp0")
                nc.vector.tensor_copy(ycp0[:, :], y0[:, :])
                ycp1 = spool.tile([65, TCH], F32, tag="ycp1")
                nc.vector.tensor_copy(ycp1[:, :], y1[:, :])
                if debug and b == 0 and g == 0:
                    nc.sync.dma_start(out=dbg["y0"][:, :], in_=ycp0[:, :])
                # den rows to base-partition-0 tiles: dual-SBUF-input DVE ops
                # (the recip NR pass) require equal base partitions
                d0 = spool.tile([1, TCH], F32, tag="d0")
                nc.vector.tensor_copy(d0[:, :], ycp0[64:65, :])
                d1 = spool.tile([1, TCH], F32, tag="d1")
                nc.vector.tensor_copy(d1[:, :], ycp1[64:65, :])
                rd0 = spool.tile([1, TCH], F32, tag="rd0")
                rs0 = spool.tile([1, TCH], F32, tag="rs0")
                nc.vector.reciprocal_approx_accurate(rd0[:, :], d0[:, :],
                                                     rs0[:, :])
                rd1 = spool.tile([1, TCH], F32, tag="rd1")
                rs1 = spool.tile([1, TCH], F32, tag="rs1")
                nc.vector.reciprocal_approx_accurate(rd1[:, :], d1[:, :],
                                                     rs1[:, :])
                r16_0 = spool.tile([1, TCH], F16, tag="r16_0")
                nc.vector.tensor_copy(r16_0[:, :], rd0[:, :])
                r16_1 = spool.tile([1, TCH], F16, tag="r16_1")
                nc.vector.tensor_copy(r16_1[:, :], rd1[:, :])

                def emit_norm():
                    rbc = mmpool.tile([128, TCH], F32, tag="mm")
                    nc.tensor.matmul(rbc[0:64, :], ones2_sb[0:1, 0:64],
                                     r16_0[:, :], start=True, stop=True)
                    nc.tensor.matmul(rbc[64:128, :], ones2_sb[0:1, 0:64],
                                     r16_1[:, :], start=True, stop=True)
                    rbc_sb0 = spool.tile([64, TCH], F32, tag="rbcs0")
                    nc.vector.tensor_copy(rbc_sb0[:, :], rbc[0:64, :])
                    rbc_sb1 = spool.tile([64, TCH], F32, tag="rbcs1")
                    nc.vector.tensor_copy(rbc_sb1[:, :], rbc[64:128, :])
                    if debug and b == 0 and g == 0:
                        nc.sync.dma_start(out=dbg["rbc"][0:64, :],
                                          in_=rbc_sb0[:, :])
                        nc.sync.dma_start(out=dbg["rbc"][64:128, :],
                                          in_=rbc_sb1[:, :])
                    y_pair = spool.tile([128, TCH], F16, tag="ypair")
                    nc.vector.scalar_tensor_tensor(
                        y_pair[0:64, :], ycp0[0:64, :], 1.0, rbc_sb0[:, :],
                        op0=mybir.AluOpType.mult, op1=mybir.AluOpType.mult)
                    nc.vector.scalar_tensor_tensor(
                        y_pair[64:128, :], ycp1[0:64, :], 1.0, rbc_sb1[:, :],
                        op0=mybir.AluOpType.mult, op1=mybir.AluOpType.mult)
                    if debug and b == 0 and g == 0:
                        nc.sync.dma_start(out=dbg["ypair"][:, :],
                                          in_=y_pair[:, :])
                    # ship to the AllToAll input buffer (2 shards)
                    for s in range(2):
                        j = 2 * g + s
                        nc.sync.dma_start(
                            out=cc_in[b][j * 128:(j + 1) * 128, :],
                            in_=y_pair[:, s * TOK:(s + 1) * TOK])
                return emit_norm

            def attn_group(b, g, pending_fin, filler=None):
                """Emit one query group; pending_fin is the previous group's
                finalize closure (emitted after our first two score pairs).
                filler, if given, is invoked once per Tt iteration to
                interleave independent tensor work into the ACT-bound
                stream."""
                ntk = 4 * g + 4
                y0 = ypool.tile([65, TCH], F32, tag="y")
                y1 = ypool.tile([65, TCH], F32, tag="y")
                pend = []
                for i in range(ntk):
                    pend.append((i, *emit_sc(b, g, i)))
                    if i == 1 and pending_fin is not None:
                        pending_fin()
                        pending_fin = None
                    if len(pend) >= 2:
                        j, c0, e = pend.pop(0)
                        emit_av(b, g, j, c0, e, y0, y1)
                    if filler is not None and i % 2 == 1:
                        filler()
                if pending_fin is not None:   # g == 0 case (ntk may be small)
                    pending_fin()
                for (j, c0, e) in pend:
                    emit_av(b, g, j, c0, e, y0, y1)
                return finish_group(b, g, (y0, y1))

            def readback(b, eng):
                """Issue A2A output readback DMAs. b=0 goes on the sync
                queue so its semaphores stay disjoint from the b=1 gpsimd
                readback (shared-semaphore thresholds otherwise coarsen
                proj(0)'s waits to include the b=1 A2A)."""
                ya = spool.tile([128, 8 * TOK], F16, tag=f"ya{b}", bufs=1)
                for c in range(8):
                    eng.dma_start(
                        out=ya[:, c * TOK:(c + 1) * TOK],
                        in_=cc_out[b][c * 128:(c + 1) * 128, :])
                if debug and b == 0:
                    nc.sync.dma_start(out=dbg["ya"][:, :], in_=ya[:, :])
                return ya

            def proj_mm(b, ya):
                """Output projection of this core's 256-token slice of b."""
                osb = spool.tile([128, 8 * TOK], F16, tag="osb")
                for o in range(8):
                    proj_strip(b, ya, osb, o)

            def proj_strip(b, ya, osb, o):
                pp = mmpool.tile([128, TOK], F32, tag="mm")
                for c in range(8):
                    nc.tensor.matmul(
                        pp[:, :],
                        wp_sb[:, c * 1024 + o * 128:c * 1024 + (o + 1) * 128],
                        ya[:, c * TOK:(c + 1) * TOK],
                        start=(c == 0), stop=(c == 7))
                nc.vector.tensor_copy(osb[:, o * TOK:(o + 1) * TOK],
                                      pp[:, :])
                nc.sync.dma_start(out=out_ext[b * 8 + o, :, :],
                                  in_=osb[:, o * TOK:(o + 1) * TOK])

            def alloc_state(b):
                rope_q = bpool.tile([128, T], F16, tag="rope_q")
                rope_k = bpool.tile([128, T], F16, tag="rope_k")
                vT = bpool.tile([128, T], F16, tag="vT")
                state[b] = (rope_q, rope_k, vT)
                va0 = bpool.tile([128, NBLK * 65], F16, tag="vaug0")
                va1 = bpool.tile([128, NBLK * 65], F16, tag="vaug1")
                nc.vector.memset(va0[:, :], 1.0)
                nc.vector.memset(va1[:, :], 1.0)
                state[(b, "vaug")] = (va0, va1)

            # ================= schedule =================
            alloc_state(0)
            # phase A: QKV b0 (+ transposes interleaved)
            for g in range(NTC):
                qkv_chunk(0, g)
                if g >= 1:
                    transposes(0, g - 1)
            transposes(0, NTC - 1)

            if debug:
                rq0, rk0, _ = state[0]
                va0_0, _ = state[(0, "vaug")]
                nc.sync.dma_start(out=dbg["rope_q"][:, :], in_=rq0[:, :])
                nc.sync.dma_start(out=dbg["rope_k"][:, :], in_=rk0[:, :])
                nc.sync.dma_start(out=dbg["vaug"][:, :], in_=va0_0[:, :])

            # phase B: attention b0 interleaved with QKV b1; the last b1
            # chunk is deferred into phase C to balance tensor work against
            # the ACT-bound b1 attention
            alloc_state(1)
            fin = None
            for g in range(NTC):
                fin = attn_group(0, g, fin)
                qkv_chunk(1, g)
                if g >= 1:
                    transposes(1, g - 1)
            fin()
            # A2A for b0 (gpsimd queue) + readback issued right behind it
            nc.gpsimd.collective_compute(
                "AllToAll", mybir.AluOpType.bypass,
                replica_groups=groups,
                ins=[cc_in[0].ap().opt()],
                outs=[cc_out[0].ap().opt()])
            ya0 = readback(0, nc.sync)

            # phase C: attention b1 (qkv chunk 3 + its transposes interleave
            # into the early ACT-bound groups); proj(b0) o-strips interleave
            # into the LAST group so they are scheduled before the b1
            # readback exists (clean rb0-based waits) and fill the ACT-bound
            # stalls there
            fin = None
            for g in range(NTC):
                fin = attn_group(1, g, fin)
                if g == 0:
                    transposes(1, NTC - 1)
            fin()
            proj_mm(0, ya0)
            nc.gpsimd.collective_compute(
                "AllToAll", mybir.AluOpType.bypass,
                replica_groups=groups,
                ins=[cc_in[1].ap().opt()],
                outs=[cc_out[1].ap().opt()])
            # b1 readback split across two queues to halve the serial
            # post-A2A DMA-issue latency
            ya1 = spool.tile([128, 8 * TOK], F16, tag="ya1", bufs=1)
            for c in range(8):
                eng = nc.gpsimd if c % 2 == 0 else nc.scalar
                eng.dma_start(out=ya1[:, c * TOK:(c + 1) * TOK],
                              in_=cc_out[1][c * 128:(c + 1) * 128, :])
            proj_mm(1, ya1)

    if not nc.is_finalized():
        nc.finalize()
    return nc


_NC_CACHE = None


def _get_nc():
    global _NC_CACHE
    if _NC_CACHE is None:
        _NC_CACHE = build()
    return _NC_CACHE


def make_in_maps(x, w_qkv, w_proj):
    x = np.asarray(x, np.float32)
    w_qkv = np.asarray(w_qkv, np.float32)
    w_proj = np.asarray(w_proj, np.float32)
    # x2[b*4+g, p, c*512+t] = x[b, g*512+t, c*128+p]
    x2 = np.ascontiguousarray(
        x.reshape(B, NTC, TCH, 8, 128).transpose(0, 1, 4, 3, 2)
    ).astype(np.float16).reshape(B * NTC, 128, 8 * TCH)
    # wp2[p, c*1024+m] = w_proj[m, c*128+p]
    wp2 = np.ascontiguousarray(
        w_proj.reshape(1024, 8, 128).transpose(2, 1, 0)
    ).astype(np.float16).reshape(128, 8 * 1024)
    in_maps = []
    for r in range(NCORE):
        ha, hb = 2 * r, 2 * r + 1
        qrows = (list(range(ha * 64, ha * 64 + 64))
                 + list(range(hb * 64, hb * 64 + 64)))
        rows = (qrows + [1024 + i for i in qrows] + [2048 + i for i in qrows])
        w_all = w_qkv[rows, :]  # [384, 1024]
        # w2[p, c*384+m] = w_all[m, c*128+p]
        w2 = np.ascontiguousarray(
            w_all.reshape(384, 8, 128).transpose(2, 1, 0)
        ).astype(np.float16).reshape(128, 8 * 384)
        in_maps.append({"x2": x2, "w2": w2, "wp2": wp2})
    return in_maps


def assemble(results):
    outT = np.zeros((B, C, T), np.float32)
    for r in range(NCORE):
        o = results[r]["out"].astype(np.float32)  # [B*8, 128, TOK]
        for b in range(B):
            outT[b, :, r * TOK:(r + 1) * TOK] = \
                o[b * 8:(b + 1) * 8].reshape(C, TOK)
    return np.ascontiguousarray(outT.transpose(0, 2, 1))


def run(x, w_qkv, w_proj, trace=False):
    nc = _get_nc()
    in_maps = make_in_maps(x, w_qkv, w_proj)
    res = run_bass_kernel_spmd(nc, in_maps, list(range(NCORE)), trace=trace)
    return assemble(res.results), res


def kernel(x, w_qkv, w_proj):
    out, _ = run(x, w_qkv, w_proj, trace=False)
    return out
